# revision 4
# baseline (speedup 1.0000x reference)
"""TRN2 Bass kernel for nn_Attention (B=2, S=2048, DIM=2048, 16 heads).

Sharding: tensor-parallel over heads — 8 cores x 2 heads each.
Each core computes q/k/v projections for its 2 heads over both batches,
causal attention, and a partial output projection (row-parallel wo).
Host sums the 8 partial outputs.

Layouts (per core):
  xT   [2048(k), 4096(s)]  = x.reshape(4096,2048).T          (replicated)
  wqT  [2048(k), 256(dq)]  = wq[head rows].T                  (sharded)
  wkT, wvT likewise; woT [256(dc), 2048(m)] = wo[:, head cols].T
  outp [2048(m), 4096(s)]  partial of out.T                   (summed on host)

All matmuls run in float32r (PE rounds operands to 11 mantissa bits, full
bf16-rate). Set _USE_F32R = False for exact-fp32 (4x slower matmuls).
"""

import sys

sys.path.insert(0, "/opt/trn_rl_repo")

import numpy as np

DIM = 2048
HEADS = 16
HD = 128
B = 2
S = 2048
SG = B * S  # 4096 global sequence (batch-major)
NCORES = 8
HPC = HEADS // NCORES  # 2 heads per core
DPC = HPC * HD  # 256 dims per core
KC = DIM // 128  # 16 contraction chunks
PC = 256  # projection s-chunk width
NPC = S // PC  # 8 proj chunks per batch
AC = 512  # attention sq-chunk width
NAC = S // AC  # 4 attention chunks per batch
ISQ = 1.0 / np.sqrt(np.float32(HD))

_USE_F32R = True

_prog_cache = {}


def _build_program():
    import concourse.bass as bass
    from concourse import bacc
    import concourse.mybir as mybir
    import concourse.tile as tile

    f32 = mybir.dt.float32
    fr = mybir.dt.float32r if _USE_F32R else mybir.dt.float32
    EXP = mybir.ActivationFunctionType.Exp
    LOG = mybir.ActivationFunctionType.Ln

    nc = bacc.Bacc()

    xT = nc.dram_tensor("xT", [DIM, SG], fr, kind="ExternalInput")
    wqT = nc.dram_tensor("wqT", [DIM, DPC], fr, kind="ExternalInput")
    wkT = nc.dram_tensor("wkT", [DIM, DPC], fr, kind="ExternalInput")
    wvT = nc.dram_tensor("wvT", [DIM, DPC], fr, kind="ExternalInput")
    woT = nc.dram_tensor("woT", [DPC, DIM], fr, kind="ExternalInput")
    m01x = nc.dram_tensor("m01x", [128, 1024], fr, kind="ExternalInput")
    onesA = nc.dram_tensor("onesA", [128, 1], fr, kind="ExternalInput")
    onesB = nc.dram_tensor("onesB", [1, 128], fr, kind="ExternalInput")
    outp = nc.dram_tensor("outp", [DIM, SG], f32, kind="ExternalOutput")

    with tile.TileContext(nc) as tc:
        with (
            tc.tile_pool(name="wpool", bufs=1) as wpool,
            tc.tile_pool(name="xpool", bufs=2) as xpool,
            tc.tile_pool(name="kv", bufs=1) as kvpool,
            tc.tile_pool(name="work", bufs=2) as work,
            tc.tile_pool(name="expool", bufs=3) as expool,
            tc.tile_pool(name="ps", bufs=1, space="PSUM") as ps,
        ):
            # --- resident constants / weights ---
            wqr = wpool.tile([128, KC, DPC], fr, tag="wqr")
            wkr = wpool.tile([128, KC, DPC], fr, tag="wkr")
            wvr = wpool.tile([128, KC, DPC], fr, tag="wvr")
            wor = wpool.tile([128, HPC, DIM], fr, tag="wor")
            m01 = wpool.tile([128, 1024], fr, tag="m01")
            onA = wpool.tile([128, 1], fr, tag="onA")
            onB = wpool.tile([1, 128], fr, tag="onB")
            for kc in range(KC):
                ksl = slice(kc * 128, (kc + 1) * 128)
                nc.sync.dma_start(wqr[:, kc, :], wqT[ksl, :])
                nc.sync.dma_start(wkr[:, kc, :], wkT[ksl, :])
                nc.sync.dma_start(wvr[:, kc, :], wvT[ksl, :])
            for dc in range(HPC):
                nc.sync.dma_start(wor[:, dc, :], woT[dc * 128 : (dc + 1) * 128, :])
            nc.sync.dma_start(m01[:], m01x[:])
            nc.sync.dma_start(onA[:], onesA[:])
            nc.sync.dma_start(onB[:], onesB[:])

            # resident per-core activations
            kTr = kvpool.tile([128, B * HPC, S], fr, tag="kTr")  # [d, bh, s]
            vr = kvpool.tile([128, B * (S // 128), DPC], fr, tag="vr")  # [s%, blk, d]

            for b in range(B):
                for j in range(NAC):
                    qTc = work.tile([128, HPC, AC], fr, tag="qTc")
                    # ---- projections for s-chunks [2j, 2j+1] of this batch ----
                    for half in range(AC // PC):
                        cl = (AC // PC) * j + half  # batch-local chunk
                        sg0 = b * S + cl * PC  # global col offset
                        xa = xpool.tile([128, KC, PC], fr, tag="xa")
                        nc.sync.dma_start(
                            xa[:],
                            xT[:, sg0 : sg0 + PC].rearrange(
                                "(kc p) s -> p kc s", p=128
                            ),
                        )
                        for h in range(HPC):
                            dsl = slice(h * 128, (h + 1) * 128)
                            pq = ps.tile([128, PC], f32, tag="pq", bufs=2)
                            for kc in range(KC):
                                nc.tensor.matmul(
                                    pq[:],
                                    wqr[:, kc, dsl],
                                    xa[:, kc, :],
                                    start=(kc == 0),
                                    stop=(kc == KC - 1),
                                )
                            nc.vector.tensor_copy(
                                qTc[:, h, half * PC : (half + 1) * PC], pq[:]
                            )
                            pk = ps.tile([128, PC], f32, tag="pq", bufs=2)
                            for kc in range(KC):
                                nc.tensor.matmul(
                                    pk[:],
                                    wkr[:, kc, dsl],
                                    xa[:, kc, :],
                                    start=(kc == 0),
                                    stop=(kc == KC - 1),
                                )
                            nc.vector.tensor_copy(
                                kTr[:, b * HPC + h, cl * PC : (cl + 1) * PC], pk[:]
                            )
                        for sb in range(PC // 128):
                            pv = ps.tile([128, DPC], f32, tag="pq", bufs=2)
                            for kc in range(KC):
                                nc.tensor.matmul(
                                    pv[:],
                                    xa[:, kc, sb * 128 : (sb + 1) * 128],
                                    wvr[:, kc, :],
                                    start=(kc == 0),
                                    stop=(kc == KC - 1),
                                )
                            vblk = b * (S // 128) + cl * (PC // 128) + sb
                            nc.vector.tensor_copy(vr[:, vblk, :], pv[:])

                    # ---- attention for (b, h, j) ----
                    uS = work.tile([128, HPC, AC], fr, tag="uS")
                    for h in range(HPC):
                        bh = b * HPC + h
                        nblocks = (j + 1) * (AC // 128)
                        nfull = j * (AC // 128)  # blocks fully below diagonal
                        U = ps.tile([128, AC], f32, tag="u", bufs=1)
                        se = ps.tile([1, AC], f32, tag="se", bufs=1)
                        g = 0
                        while g * 2 < nblocks:
                            i0 = g * 2
                            sc = ps.tile([128, 2, AC], f32, tag="sc", bufs=2)
                            ex = expool.tile([128, 2, AC], fr, tag="ex", bufs=2)
                            for gi in range(2):
                                i = i0 + gi
                                loc = max(0, 128 * i - AC * j)
                                nc.tensor.matmul(
                                    sc[:, gi, loc:AC],
                                    kTr[:, bh, i * 128 : (i + 1) * 128],
                                    qTc[:, h, loc:AC],
                                    start=True,
                                    stop=True,
                                )
                            if i0 + 1 < nfull:
                                # both blocks full: one exp over both banks
                                nc.scalar.activation(
                                    ex[:, :, :], sc[:, :, :], EXP, scale=ISQ
                                )
                            else:
                                for gi in range(2):
                                    i = i0 + gi
                                    loc = max(0, 128 * i - AC * j)
                                    if i < nfull:
                                        nc.scalar.activation(
                                            ex[:, gi, :], sc[:, gi, :], EXP, scale=ISQ
                                        )
                                    else:
                                        p = i - nfull
                                        ds = expool.tile([128, AC], fr, tag="ds", bufs=2)
                                        nc.scalar.activation(
                                            ds[:, loc:AC],
                                            sc[:, gi, loc:AC],
                                            EXP,
                                            scale=ISQ,
                                        )
                                        nc.vector.tensor_mul(
                                            ex[:, gi, loc:AC],
                                            ds[:, loc:AC],
                                            m01[:, 384 : 384 + AC - loc],
                                        )
                            for gi in range(2):
                                i = i0 + gi
                                loc = max(0, 128 * i - AC * j)
                                vblk = b * (S // 128) + i
                                nc.tensor.matmul(
                                    U[:, loc:AC],
                                    vr[:, vblk, h * 128 : (h + 1) * 128],
                                    ex[:, gi, loc:AC],
                                    start=(i == 0),
                                    stop=(i == nblocks - 1),
                                )
                                nc.tensor.matmul(
                                    se[:, loc:AC],
                                    onA[:],
                                    ex[:, gi, loc:AC],
                                    start=(i == 0),
                                    stop=(i == nblocks - 1),
                                )
                            g += 1
                        # normalize: uS = U * exp(-ln(sumexp)) broadcast over d
                        lnz = work.tile([1, AC], fr, tag="lnz")
                        nc.scalar.activation(lnz[:], se[:], LOG)
                        bc = ps.tile([128, 2, AC], f32, tag="sc", bufs=2)
                        nc.tensor.matmul(
                            bc[:, 0, :], onB[:], lnz[:], start=True, stop=True
                        )
                        rb = work.tile([128, AC], f32, tag="rb")
                        nc.scalar.activation(rb[:], bc[:, 0, :], EXP, scale=-1.0)
                        nc.vector.tensor_mul(uS[:, h, :], U[:], rb[:])

                    # ---- partial out-projection for (b, j) ----
                    sg0 = b * S + j * AC
                    for mb in range(DIM // 128):
                        pot = ps.tile([128, 2, AC], f32, tag="sc", bufs=2)
                        po = pot[:, 0, :]
                        for dc in range(HPC):
                            nc.tensor.matmul(
                                po,
                                wor[:, dc, mb * 128 : (mb + 1) * 128],
                                uS[:, dc, :],
                                start=(dc == 0),
                                stop=(dc == HPC - 1),
                            )
                        ob = work.tile([128, AC], f32, tag="ob")
                        if mb % 2 == 0:
                            nc.vector.tensor_copy(ob[:], po)
                        else:
                            nc.scalar.copy(ob[:], po)
                        nc.sync.dma_start(
                            outp[mb * 128 : (mb + 1) * 128, sg0 : sg0 + AC], ob[:]
                        )

    nc.finalize()
    return nc


def _get_program():
    key = "prog"
    if key not in _prog_cache:
        _prog_cache[key] = _build_program()
    return _prog_cache[key]


def _is_causal_neg_mask(mask):
    m = mask.reshape(S, S)
    tri = np.triu(np.ones((S, S), dtype=bool), k=1)
    return (
        np.all(m[~tri] == 0.0)
        and np.all(m[tri] <= -1e8)
        and np.all(np.isfinite(m) | tri)
    )


def _reference_fallback(x, mask, wq, wk, wv, wo):
    xf = x.astype(np.float32)
    q = (xf @ wq.T).reshape(B, S, HEADS, HD).transpose(0, 2, 1, 3)
    k = (xf @ wk.T).reshape(B, S, HEADS, HD).transpose(0, 2, 1, 3)
    v = (xf @ wv.T).reshape(B, S, HEADS, HD).transpose(0, 2, 1, 3)
    scores = np.einsum("bhqd,bhkd->bhqk", q, k) / np.sqrt(np.float32(HD))
    scores = scores + mask
    scores = scores - scores.max(axis=-1, keepdims=True)
    e = np.exp(scores)
    probs = e / e.sum(axis=-1, keepdims=True)
    out = np.einsum("bhqk,bhkd->bhqd", probs, v)
    out = out.transpose(0, 2, 1, 3).reshape(B, S, HEADS * HD)
    return (out @ wo.T).astype(np.float32)


def kernel(x, mask, wq, wk, wv, wo):
    x = np.ascontiguousarray(np.asarray(x, dtype=np.float32))
    mask = np.asarray(mask, dtype=np.float32)
    wq = np.ascontiguousarray(np.asarray(wq, dtype=np.float32))
    wk = np.ascontiguousarray(np.asarray(wk, dtype=np.float32))
    wv = np.ascontiguousarray(np.asarray(wv, dtype=np.float32))
    wo = np.ascontiguousarray(np.asarray(wo, dtype=np.float32))

    if not _is_causal_neg_mask(mask):
        return _reference_fallback(x, mask, wq, wk, wv, wo)

    from concourse.bass_utils import run_bass_kernel_spmd

    nc = _get_program()

    xT = np.ascontiguousarray(x.reshape(SG, DIM).T)
    # m01big[k, c] = 1.0 iff (c - 384) >= k; partial blocks slice [384:384+N)
    kk = np.arange(128)[:, None]
    cc = np.arange(1024)[None, :]
    m01x = ((cc - 384) >= kk).astype(np.float32)
    onesA = np.ones((128, 1), dtype=np.float32)
    onesB = np.ones((1, 128), dtype=np.float32)

    in_maps = []
    for c in range(NCORES):
        hs = slice(c * DPC, (c + 1) * DPC)
        in_maps.append(
            {
                "xT": xT,
                "wqT": np.ascontiguousarray(wq[hs, :].T),
                "wkT": np.ascontiguousarray(wk[hs, :].T),
                "wvT": np.ascontiguousarray(wv[hs, :].T),
                "woT": np.ascontiguousarray(wo[:, hs].T),
                "m01x": m01x,
                "onesA": onesA,
                "onesB": onesB,
            }
        )

    res = run_bass_kernel_spmd(nc, in_maps, list(range(NCORES)))
    global LAST_RESULT
    LAST_RESULT = res
    acc = res.results[0]["outp"].astype(np.float32)
    for c in range(1, NCORES):
        acc += res.results[c]["outp"]
    # outp is out.T: [m, s_glob] -> [B, S, DIM]
    return np.ascontiguousarray(acc.T).reshape(B, S, DIM)


if __name__ == "__main__":
    rng = np.random.default_rng(0)
    x = rng.standard_normal((B, S, DIM), dtype=np.float32)
    neg = np.float32(-1e9)
    maskm = np.triu(np.full((S, S), neg, dtype=np.float32), k=1)[None, None]
    ws = [rng.standard_normal((DIM, DIM), dtype=np.float32) * 0.02 for _ in range(4)]
    out = kernel(x, maskm, *ws)
    print(out.shape, out.dtype)


# revision 6
# speedup vs baseline: 1.1468x; 1.1468x over previous
"""TRN2 Bass kernel for nn_Attention (B=2, S=2048, DIM=2048, 16 heads).

Sharding: tensor-parallel over heads — 8 cores x 2 heads each.
Each core computes q/k/v projections for its 2 heads over both batches,
causal attention, and a partial output projection (row-parallel wo).
Host sums the 8 partial outputs.

Layouts (per core):
  xT   [2048(k), 4096(s)]  = x.reshape(4096,2048).T          (replicated)
  wqT  [2048(k), 256(dq)]  = wq[head rows].T                  (sharded)
  wkT, wvT likewise; woT [256(dc), 2048(m)] = wo[:, head cols].T
  outp [2048(m), 4096(s)]  partial of out.T                   (summed on host)

All matmuls run in float32r (PE rounds operands to 11 mantissa bits, full
bf16-rate). Set _USE_F32R = False for exact-fp32 (4x slower matmuls).
"""

import sys

sys.path.insert(0, "/opt/trn_rl_repo")

import numpy as np

DIM = 2048
HEADS = 16
HD = 128
B = 2
S = 2048
SG = B * S  # 4096 global sequence (batch-major)
NCORES = 8
HPC = HEADS // NCORES  # 2 heads per core
DPC = HPC * HD  # 256 dims per core
KC = DIM // 128  # 16 contraction chunks
PC = 256  # projection s-chunk width
NPC = S // PC  # 8 proj chunks per batch
AC = 512  # attention sq-chunk width
NAC = S // AC  # 4 attention chunks per batch
ISQ = 1.0 / np.sqrt(np.float32(HD))

_USE_F32R = True

_prog_cache = {}


def _build_program():
    import concourse.bass as bass
    from concourse import bacc
    import concourse.mybir as mybir
    import concourse.tile as tile

    f32 = mybir.dt.float32
    fr = mybir.dt.float32r if _USE_F32R else mybir.dt.float32
    EXP = mybir.ActivationFunctionType.Exp
    LOG = mybir.ActivationFunctionType.Ln

    nc = bacc.Bacc()

    xT = nc.dram_tensor("xT", [DIM, SG], fr, kind="ExternalInput")
    wqT = nc.dram_tensor("wqT", [DIM, DPC], fr, kind="ExternalInput")
    wkT = nc.dram_tensor("wkT", [DIM, DPC], fr, kind="ExternalInput")
    wvT = nc.dram_tensor("wvT", [DIM, DPC], fr, kind="ExternalInput")
    woT = nc.dram_tensor("woT", [DPC, DIM], fr, kind="ExternalInput")
    m01x = nc.dram_tensor("m01x", [128, 1024], fr, kind="ExternalInput")
    onesA = nc.dram_tensor("onesA", [128, 1], fr, kind="ExternalInput")
    onesB = nc.dram_tensor("onesB", [1, 128], fr, kind="ExternalInput")
    outp = nc.dram_tensor("outp", [DIM, SG], f32, kind="ExternalOutput")

    with tile.TileContext(nc) as tc:
        with (
            tc.tile_pool(name="wpool", bufs=1) as wpool,
            tc.tile_pool(name="xpool", bufs=2) as xpool,
            tc.tile_pool(name="kv", bufs=1) as kvpool,
            tc.tile_pool(name="work", bufs=2) as work,
            tc.tile_pool(name="expool", bufs=3) as expool,
            tc.tile_pool(name="ps", bufs=1, space="PSUM") as ps,
        ):
            # --- resident constants / weights ---
            wqr = wpool.tile([128, KC, DPC], fr, tag="wqr")
            wkr = wpool.tile([128, KC, DPC], fr, tag="wkr")
            wvr = wpool.tile([128, KC, DPC], fr, tag="wvr")
            wor = wpool.tile([128, HPC, DIM], fr, tag="wor")
            m01 = wpool.tile([128, 1024], fr, tag="m01")
            onA = wpool.tile([128, 1], fr, tag="onA")
            onB = wpool.tile([1, 128], fr, tag="onB")
            for kc in range(KC):
                ksl = slice(kc * 128, (kc + 1) * 128)
                nc.sync.dma_start(wqr[:, kc, :], wqT[ksl, :])
                nc.sync.dma_start(wkr[:, kc, :], wkT[ksl, :])
                nc.sync.dma_start(wvr[:, kc, :], wvT[ksl, :])
            for dc in range(HPC):
                nc.sync.dma_start(wor[:, dc, :], woT[dc * 128 : (dc + 1) * 128, :])
            nc.sync.dma_start(m01[:], m01x[:])
            nc.sync.dma_start(onA[:], onesA[:])
            nc.sync.dma_start(onB[:], onesB[:])

            # resident per-core activations
            kTr = kvpool.tile([128, B * HPC, S], fr, tag="kTr")  # [d, bh, s]
            vr = kvpool.tile([128, B * (S // 128), DPC], fr, tag="vr")  # [s%, blk, d]

            def emit_proj(b, j, qTc):
                for half in range(AC // PC):
                    cl = (AC // PC) * j + half
                    sg0 = b * S + cl * PC
                    xa = xpool.tile([128, KC, PC], fr, tag="xa")
                    nc.sync.dma_start(
                        xa[:],
                        xT[:, sg0 : sg0 + PC].rearrange("(kc p) s -> p kc s", p=128),
                    )
                    for h in range(HPC):
                        dsl = slice(h * 128, (h + 1) * 128)
                        pq = ps.tile([128, PC], f32, tag="pq", bufs=2)
                        for kc in range(KC):
                            nc.tensor.matmul(
                                pq[:],
                                wqr[:, kc, dsl],
                                xa[:, kc, :],
                                start=(kc == 0),
                                stop=(kc == KC - 1),
                            )
                        nc.vector.tensor_copy(
                            qTc[:, h, half * PC : (half + 1) * PC], pq[:]
                        )
                        pk = ps.tile([128, PC], f32, tag="pq", bufs=2)
                        for kc in range(KC):
                            nc.tensor.matmul(
                                pk[:],
                                wkr[:, kc, dsl],
                                xa[:, kc, :],
                                start=(kc == 0),
                                stop=(kc == KC - 1),
                            )
                        nc.vector.tensor_copy(
                            kTr[:, b * HPC + h, cl * PC : (cl + 1) * PC], pk[:]
                        )
                    for sb in range(PC // 128):
                        pv = ps.tile([128, DPC], f32, tag="pq", bufs=2)
                        for kc in range(KC):
                            nc.tensor.matmul(
                                pv[:],
                                xa[:, kc, sb * 128 : (sb + 1) * 128],
                                wvr[:, kc, :],
                                start=(kc == 0),
                                stop=(kc == KC - 1),
                            )
                        vblk = b * (S // 128) + cl * (PC // 128) + sb
                        nc.vector.tensor_copy(vr[:, vblk, :], pv[:])

            def emit_attention(b, j, qTc, uS):
                for h in range(HPC):
                    bh = b * HPC + h
                    nblocks = (j + 1) * (AC // 128)
                    nfull = j * (AC // 128)
                    U = ps.tile([128, AC], f32, tag="u", bufs=2)
                    se = ps.tile([1, AC], f32, tag="se", bufs=2)
                    for i in range(nblocks):
                        loc = max(0, 128 * i - AC * j)
                        sc = ps.tile([128, AC], f32, tag="sc", bufs=2)
                        ex = expool.tile([128, AC], fr, tag="ex", bufs=3)
                        nc.tensor.matmul(
                            sc[:, loc:AC],
                            kTr[:, bh, i * 128 : (i + 1) * 128],
                            qTc[:, h, loc:AC],
                            start=True,
                            stop=True,
                        )
                        if i < nfull:
                            nc.scalar.activation(ex[:], sc[:], EXP, scale=ISQ)
                        else:
                            ds = expool.tile([128, AC], fr, tag="ds", bufs=2)
                            nc.scalar.activation(
                                ds[:, loc:AC], sc[:, loc:AC], EXP, scale=ISQ
                            )
                            nc.vector.tensor_mul(
                                ex[:, loc:AC],
                                ds[:, loc:AC],
                                m01[:, 384 : 384 + AC - loc],
                            )
                        vblk = b * (S // 128) + i
                        nc.tensor.matmul(
                            U[:, loc:AC],
                            vr[:, vblk, h * 128 : (h + 1) * 128],
                            ex[:, loc:AC],
                            start=(i == 0),
                            stop=(i == nblocks - 1),
                        )
                        nc.tensor.matmul(
                            se[:, loc:AC],
                            onA[:],
                            ex[:, loc:AC],
                            start=(i == 0),
                            stop=(i == nblocks - 1),
                        )
                    # normalize: uS = U * exp(-ln(sumexp)) broadcast over d
                    lnz = work.tile([1, AC], fr, tag="lnz")
                    nc.scalar.activation(lnz[:], se[:], LOG)
                    bc = ps.tile([128, AC], f32, tag="sc", bufs=2)
                    nc.tensor.matmul(bc[:], onB[:], lnz[:], start=True, stop=True)
                    rb = work.tile([128, AC], f32, tag="rb")
                    nc.scalar.activation(rb[:], bc[:], EXP, scale=-1.0)
                    nc.vector.tensor_mul(uS[:, h, :], U[:], rb[:])

            def emit_outproj(b, j, uS):
                sg0 = b * S + j * AC
                for mb in range(DIM // 128):
                    po = ps.tile([128, AC], f32, tag="sc", bufs=2)
                    for dc in range(HPC):
                        nc.tensor.matmul(
                            po[:],
                            wor[:, dc, mb * 128 : (mb + 1) * 128],
                            uS[:, dc, :],
                            start=(dc == 0),
                            stop=(dc == HPC - 1),
                        )
                    ob = work.tile([128, AC], f32, tag="ob")
                    if mb % 2 == 0:
                        nc.vector.tensor_copy(ob[:], po[:])
                    else:
                        nc.scalar.copy(ob[:], po[:])
                    nc.sync.dma_start(
                        outp[mb * 128 : (mb + 1) * 128, sg0 : sg0 + AC], ob[:]
                    )

            # software-pipelined emission: projections run one chunk ahead so
            # dense PE work overlaps attention's ACT-wait bubbles
            chunks = [(b, j) for b in range(B) for j in range(NAC)]
            qTcs = {}
            qTcs[chunks[0]] = work.tile([128, HPC, AC], fr, tag="qTc", name="qTc0")
            emit_proj(*chunks[0], qTcs[chunks[0]])
            for idx, (b, j) in enumerate(chunks):
                if idx + 1 < len(chunks):
                    nb, nj = chunks[idx + 1]
                    qTcs[(nb, nj)] = work.tile([128, HPC, AC], fr, tag="qTc", name=f"qTc_{nb}_{nj}")
                    emit_proj(nb, nj, qTcs[(nb, nj)])
                uS = work.tile([128, HPC, AC], fr, tag="uS")
                emit_attention(b, j, qTcs.pop((b, j)), uS)
                emit_outproj(b, j, uS)

    nc.finalize()
    return nc


def _get_program():
    key = "prog"
    if key not in _prog_cache:
        _prog_cache[key] = _build_program()
    return _prog_cache[key]


def _is_causal_neg_mask(mask):
    m = mask.reshape(S, S)
    tri = np.triu(np.ones((S, S), dtype=bool), k=1)
    return (
        np.all(m[~tri] == 0.0)
        and np.all(m[tri] <= -1e8)
        and np.all(np.isfinite(m) | tri)
    )


def _reference_fallback(x, mask, wq, wk, wv, wo):
    xf = x.astype(np.float32)
    q = (xf @ wq.T).reshape(B, S, HEADS, HD).transpose(0, 2, 1, 3)
    k = (xf @ wk.T).reshape(B, S, HEADS, HD).transpose(0, 2, 1, 3)
    v = (xf @ wv.T).reshape(B, S, HEADS, HD).transpose(0, 2, 1, 3)
    scores = np.einsum("bhqd,bhkd->bhqk", q, k) / np.sqrt(np.float32(HD))
    scores = scores + mask
    scores = scores - scores.max(axis=-1, keepdims=True)
    e = np.exp(scores)
    probs = e / e.sum(axis=-1, keepdims=True)
    out = np.einsum("bhqk,bhkd->bhqd", probs, v)
    out = out.transpose(0, 2, 1, 3).reshape(B, S, HEADS * HD)
    return (out @ wo.T).astype(np.float32)


def kernel(x, mask, wq, wk, wv, wo):
    x = np.ascontiguousarray(np.asarray(x, dtype=np.float32))
    mask = np.asarray(mask, dtype=np.float32)
    wq = np.ascontiguousarray(np.asarray(wq, dtype=np.float32))
    wk = np.ascontiguousarray(np.asarray(wk, dtype=np.float32))
    wv = np.ascontiguousarray(np.asarray(wv, dtype=np.float32))
    wo = np.ascontiguousarray(np.asarray(wo, dtype=np.float32))

    if not _is_causal_neg_mask(mask):
        return _reference_fallback(x, mask, wq, wk, wv, wo)

    from concourse.bass_utils import run_bass_kernel_spmd

    nc = _get_program()

    xT = np.ascontiguousarray(x.reshape(SG, DIM).T)
    # m01big[k, c] = 1.0 iff (c - 384) >= k; partial blocks slice [384:384+N)
    kk = np.arange(128)[:, None]
    cc = np.arange(1024)[None, :]
    m01x = ((cc - 384) >= kk).astype(np.float32)
    onesA = np.ones((128, 1), dtype=np.float32)
    onesB = np.ones((1, 128), dtype=np.float32)

    in_maps = []
    for c in range(NCORES):
        hs = slice(c * DPC, (c + 1) * DPC)
        in_maps.append(
            {
                "xT": xT,
                "wqT": np.ascontiguousarray(wq[hs, :].T),
                "wkT": np.ascontiguousarray(wk[hs, :].T),
                "wvT": np.ascontiguousarray(wv[hs, :].T),
                "woT": np.ascontiguousarray(wo[:, hs].T),
                "m01x": m01x,
                "onesA": onesA,
                "onesB": onesB,
            }
        )

    res = run_bass_kernel_spmd(nc, in_maps, list(range(NCORES)))
    global LAST_RESULT
    LAST_RESULT = res
    acc = res.results[0]["outp"].astype(np.float32)
    for c in range(1, NCORES):
        acc += res.results[c]["outp"]
    # outp is out.T: [m, s_glob] -> [B, S, DIM]
    return np.ascontiguousarray(acc.T).reshape(B, S, DIM)


if __name__ == "__main__":
    rng = np.random.default_rng(0)
    x = rng.standard_normal((B, S, DIM), dtype=np.float32)
    neg = np.float32(-1e9)
    maskm = np.triu(np.full((S, S), neg, dtype=np.float32), k=1)[None, None]
    ws = [rng.standard_normal((DIM, DIM), dtype=np.float32) * 0.02 for _ in range(4)]
    out = kernel(x, maskm, *ws)
    print(out.shape, out.dtype)


# revision 7
# speedup vs baseline: 1.4080x; 1.2278x over previous
"""TRN2 Bass kernel for nn_Attention (B=2, S=2048, DIM=2048, 16 heads).

Sharding: tensor-parallel over heads — 8 cores x 2 heads each.
Each core computes q/k/v projections for its 2 heads over both batches,
causal attention, and a partial output projection (row-parallel wo).
Host sums the 8 partial outputs.

Layouts (per core):
  xT   [2048(k), 4096(s)]  = x.reshape(4096,2048).T          (replicated)
  wqT  [2048(k), 256(dq)]  = wq[head rows].T                  (sharded)
  wkT, wvT likewise; woT [256(dc), 2048(m)] = wo[:, head cols].T
  outp [2048(m), 4096(s)]  partial of out.T                   (summed on host)

All matmuls run in float32r (PE rounds operands to 11 mantissa bits, full
bf16-rate). Set _USE_F32R = False for exact-fp32 (4x slower matmuls).
"""

import sys

sys.path.insert(0, "/opt/trn_rl_repo")

import numpy as np

DIM = 2048
HEADS = 16
HD = 128
B = 2
S = 2048
SG = B * S  # 4096 global sequence (batch-major)
NCORES = 8
HPC = HEADS // NCORES  # 2 heads per core
DPC = HPC * HD  # 256 dims per core
KC = DIM // 128  # 16 contraction chunks
PC = 256  # projection s-chunk width
NPC = S // PC  # 8 proj chunks per batch
AC = 512  # attention sq-chunk width
NAC = S // AC  # 4 attention chunks per batch
ISQ = 1.0 / np.sqrt(np.float32(HD))

_USE_F32R = True

_prog_cache = {}


def _build_program():
    import concourse.bass as bass
    from concourse import bacc
    import concourse.mybir as mybir
    import concourse.tile as tile

    f32 = mybir.dt.float32
    fr = mybir.dt.float32r if _USE_F32R else mybir.dt.float32
    EXP = mybir.ActivationFunctionType.Exp
    LOG = mybir.ActivationFunctionType.Ln

    nc = bacc.Bacc()

    xT = nc.dram_tensor("xT", [DIM, SG], fr, kind="ExternalInput")
    wqT = nc.dram_tensor("wqT", [DIM, DPC], fr, kind="ExternalInput")
    wkT = nc.dram_tensor("wkT", [DIM, DPC], fr, kind="ExternalInput")
    wvT = nc.dram_tensor("wvT", [DIM, DPC], fr, kind="ExternalInput")
    woT = nc.dram_tensor("woT", [DPC, DIM], fr, kind="ExternalInput")
    m01x = nc.dram_tensor("m01x", [128, 1024], fr, kind="ExternalInput")
    onesA = nc.dram_tensor("onesA", [128, 1], fr, kind="ExternalInput")
    onesB = nc.dram_tensor("onesB", [1, 128], fr, kind="ExternalInput")
    outp = nc.dram_tensor("outp", [DIM, SG], f32, kind="ExternalOutput")

    with tile.TileContext(nc) as tc:
        with (
            tc.tile_pool(name="wpool", bufs=1) as wpool,
            tc.tile_pool(name="xpool", bufs=2) as xpool,
            tc.tile_pool(name="kv", bufs=1) as kvpool,
            tc.tile_pool(name="work", bufs=2) as work,
            tc.tile_pool(name="expool", bufs=3) as expool,
            tc.tile_pool(name="ps", bufs=1, space="PSUM") as ps,
        ):
            # --- resident constants / weights ---
            wqr = wpool.tile([128, KC, DPC], fr, tag="wqr")
            wkr = wpool.tile([128, KC, DPC], fr, tag="wkr")
            wvr = wpool.tile([128, KC, DPC], fr, tag="wvr")
            wor = wpool.tile([128, HPC, DIM], fr, tag="wor")
            m01 = wpool.tile([128, 1024], fr, tag="m01")
            onA = wpool.tile([128, 1], fr, tag="onA")
            onB = wpool.tile([1, 128], fr, tag="onB")
            for kc in range(KC):
                ksl = slice(kc * 128, (kc + 1) * 128)
                nc.sync.dma_start(wqr[:, kc, :], wqT[ksl, :])
                nc.sync.dma_start(wkr[:, kc, :], wkT[ksl, :])
                nc.sync.dma_start(wvr[:, kc, :], wvT[ksl, :])
            for dc in range(HPC):
                nc.sync.dma_start(wor[:, dc, :], woT[dc * 128 : (dc + 1) * 128, :])
            nc.sync.dma_start(m01[:], m01x[:])
            nc.sync.dma_start(onA[:], onesA[:])
            nc.sync.dma_start(onB[:], onesB[:])

            # resident per-core activations
            kTr = kvpool.tile([128, B * HPC, S], fr, tag="kTr")  # [d, bh, s]
            vr = kvpool.tile([128, B * (S // 128), DPC], fr, tag="vr")  # [s%, blk, d]

            def proj_units(b, j, qTc):
                units = []
                for half in range(AC // PC):
                    cl = (AC // PC) * j + half
                    sg0 = b * S + cl * PC
                    xa = xpool.tile(
                        [128, KC, PC], fr, tag="xa", name=f"xa_{b}_{j}_{half}"
                    )

                    def dma_unit(xa=xa, sg0=sg0):
                        nc.sync.dma_start(
                            xa[:],
                            xT[:, sg0 : sg0 + PC].rearrange(
                                "(kc p) s -> p kc s", p=128
                            ),
                        )

                    units.append(dma_unit)
                    for h in range(HPC):
                        def q_unit(h=h, xa=xa, half=half):
                            dsl = slice(h * 128, (h + 1) * 128)
                            pq = ps.tile([128, PC], f32, tag="pq", bufs=2)
                            for kc in range(KC):
                                nc.tensor.matmul(
                                    pq[:], wqr[:, kc, dsl], xa[:, kc, :],
                                    start=(kc == 0), stop=(kc == KC - 1),
                                )
                            nc.vector.tensor_copy(
                                qTc[:, h, half * PC : (half + 1) * PC], pq[:]
                            )

                        def k_unit(h=h, xa=xa, cl=cl):
                            dsl = slice(h * 128, (h + 1) * 128)
                            pk = ps.tile([128, PC], f32, tag="pq", bufs=2)
                            for kc in range(KC):
                                nc.tensor.matmul(
                                    pk[:], wkr[:, kc, dsl], xa[:, kc, :],
                                    start=(kc == 0), stop=(kc == KC - 1),
                                )
                            nc.vector.tensor_copy(
                                kTr[:, b * HPC + h, cl * PC : (cl + 1) * PC], pk[:]
                            )

                        units.append(q_unit)
                        units.append(k_unit)
                    for sb in range(PC // 128):
                        def v_unit(sb=sb, xa=xa, cl=cl):
                            pv = ps.tile([128, DPC], f32, tag="pq", bufs=2)
                            for kc in range(KC):
                                nc.tensor.matmul(
                                    pv[:], xa[:, kc, sb * 128 : (sb + 1) * 128],
                                    wvr[:, kc, :],
                                    start=(kc == 0), stop=(kc == KC - 1),
                                )
                            vblk = b * (S // 128) + cl * (PC // 128) + sb
                            nc.vector.tensor_copy(vr[:, vblk, :], pv[:])

                        units.append(v_unit)
                return units

            def att_units(b, j, qTc, uS):
                units = []
                for h in range(HPC):
                    bh = b * HPC + h
                    nblocks = (j + 1) * (AC // 128)
                    nfull = j * (AC // 128)
                    box = {}

                    def head_start(box=box, h=h):
                        box["U"] = ps.tile([128, AC], f32, tag="u", bufs=2,
                                           name=f"U_{b}_{j}_{h}")
                        box["se"] = ps.tile([1, AC], f32, tag="se", bufs=1,
                                            name=f"se_{b}_{j}_{h}")

                    for i in range(nblocks):
                        def block_unit(i=i, h=h, bh=bh, box=box,
                                       nblocks=nblocks, nfull=nfull):
                            if i == 0:
                                head_start(box, h)
                            U, se = box["U"], box["se"]
                            loc = max(0, 128 * i - AC * j)
                            sc = ps.tile([128, AC], f32, tag="sc", bufs=2)
                            ex = expool.tile([128, AC], fr, tag="ex", bufs=4)
                            nc.tensor.matmul(
                                sc[:, loc:AC],
                                kTr[:, bh, i * 128 : (i + 1) * 128],
                                qTc[:, h, loc:AC],
                                start=True, stop=True,
                            )
                            if i < nfull:
                                nc.scalar.activation(ex[:], sc[:], EXP, scale=ISQ)
                            else:
                                ds = expool.tile([128, AC], fr, tag="ds", bufs=2)
                                nc.scalar.activation(
                                    ds[:, loc:AC], sc[:, loc:AC], EXP, scale=ISQ
                                )
                                nc.vector.tensor_mul(
                                    ex[:, loc:AC], ds[:, loc:AC],
                                    m01[:, 384 : 384 + AC - loc],
                                )
                            vblk = b * (S // 128) + i
                            nc.tensor.matmul(
                                U[:, loc:AC],
                                vr[:, vblk, h * 128 : (h + 1) * 128],
                                ex[:, loc:AC],
                                start=(i == 0), stop=(i == nblocks - 1),
                            )
                            nc.tensor.matmul(
                                se[:, loc:AC], onA[:], ex[:, loc:AC],
                                start=(i == 0), stop=(i == nblocks - 1),
                            )

                        units.append(block_unit)

                    def norm_unit(h=h, box=box):
                        U, se = box["U"], box["se"]
                        lnz = work.tile([1, AC], fr, tag="lnz")
                        nc.scalar.activation(lnz[:], se[:], LOG)
                        bc = ps.tile([128, AC], f32, tag="sc", bufs=2)
                        nc.tensor.matmul(bc[:], onB[:], lnz[:], start=True, stop=True)
                        rb = work.tile([128, AC], f32, tag="rb")
                        nc.scalar.activation(rb[:], bc[:], EXP, scale=-1.0)
                        nc.vector.tensor_mul(uS[:, h, :], U[:], rb[:])

                    units.append(norm_unit)
                return units

            def out_units(b, j, uS):
                units = []
                sg0 = b * S + j * AC
                for mb in range(DIM // 128):
                    def o_unit(mb=mb):
                        po = ps.tile([128, AC], f32, tag="po", bufs=1)
                        for dc in range(HPC):
                            nc.tensor.matmul(
                                po[:],
                                wor[:, dc, mb * 128 : (mb + 1) * 128],
                                uS[:, dc, :],
                                start=(dc == 0), stop=(dc == HPC - 1),
                            )
                        ob = work.tile([128, AC], f32, tag="ob")
                        if mb % 2 == 0:
                            nc.vector.tensor_copy(ob[:], po[:])
                        else:
                            nc.scalar.copy(ob[:], po[:])
                        nc.sync.dma_start(
                            outp[mb * 128 : (mb + 1) * 128, sg0 : sg0 + AC], ob[:]
                        )

                    units.append(o_unit)
                return units

            def merge_emit(a_units, b_units):
                na, nb = len(a_units), len(b_units)
                ia = ib = 0
                while ia < na or ib < nb:
                    fa = ia / na if na else 2.0
                    fb = ib / nb if nb else 2.0
                    if fa <= fb:
                        a_units[ia]()
                        ia += 1
                    else:
                        b_units[ib]()
                        ib += 1

            # software pipeline: att(c) interleaved with proj(c+1) + out(c-1)
            chunks = [(b, j) for b in range(B) for j in range(NAC)]
            qTcs = {}
            uSs = {}
            qTcs[chunks[0]] = work.tile([128, HPC, AC], fr, tag="qTc", name="qTc0")
            for u in proj_units(*chunks[0], qTcs[chunks[0]]):
                u()
            for idx, (b, j) in enumerate(chunks):
                fill = []
                if idx + 1 < len(chunks):
                    nb_, nj_ = chunks[idx + 1]
                    qTcs[(nb_, nj_)] = work.tile(
                        [128, HPC, AC], fr, tag="qTc", name=f"qTc_{nb_}_{nj_}"
                    )
                    fill += proj_units(nb_, nj_, qTcs[(nb_, nj_)])
                if idx > 0:
                    fill += out_units(*chunks[idx - 1], uSs.pop(chunks[idx - 1]))
                uS = work.tile([128, HPC, AC], fr, tag="uS", name=f"uS_{b}_{j}")
                uSs[(b, j)] = uS
                merge_emit(att_units(b, j, qTcs.pop((b, j)), uS), fill)
            for u in out_units(*chunks[-1], uSs.pop(chunks[-1])):
                u()

    nc.finalize()
    return nc


def _get_program():
    key = "prog"
    if key not in _prog_cache:
        _prog_cache[key] = _build_program()
    return _prog_cache[key]


def _is_causal_neg_mask(mask):
    m = mask.reshape(S, S)
    tri = np.triu(np.ones((S, S), dtype=bool), k=1)
    return (
        np.all(m[~tri] == 0.0)
        and np.all(m[tri] <= -1e8)
        and np.all(np.isfinite(m) | tri)
    )


def _reference_fallback(x, mask, wq, wk, wv, wo):
    xf = x.astype(np.float32)
    q = (xf @ wq.T).reshape(B, S, HEADS, HD).transpose(0, 2, 1, 3)
    k = (xf @ wk.T).reshape(B, S, HEADS, HD).transpose(0, 2, 1, 3)
    v = (xf @ wv.T).reshape(B, S, HEADS, HD).transpose(0, 2, 1, 3)
    scores = np.einsum("bhqd,bhkd->bhqk", q, k) / np.sqrt(np.float32(HD))
    scores = scores + mask
    scores = scores - scores.max(axis=-1, keepdims=True)
    e = np.exp(scores)
    probs = e / e.sum(axis=-1, keepdims=True)
    out = np.einsum("bhqk,bhkd->bhqd", probs, v)
    out = out.transpose(0, 2, 1, 3).reshape(B, S, HEADS * HD)
    return (out @ wo.T).astype(np.float32)


def kernel(x, mask, wq, wk, wv, wo):
    x = np.ascontiguousarray(np.asarray(x, dtype=np.float32))
    mask = np.asarray(mask, dtype=np.float32)
    wq = np.ascontiguousarray(np.asarray(wq, dtype=np.float32))
    wk = np.ascontiguousarray(np.asarray(wk, dtype=np.float32))
    wv = np.ascontiguousarray(np.asarray(wv, dtype=np.float32))
    wo = np.ascontiguousarray(np.asarray(wo, dtype=np.float32))

    if not _is_causal_neg_mask(mask):
        return _reference_fallback(x, mask, wq, wk, wv, wo)

    from concourse.bass_utils import run_bass_kernel_spmd

    nc = _get_program()

    xT = np.ascontiguousarray(x.reshape(SG, DIM).T)
    # m01big[k, c] = 1.0 iff (c - 384) >= k; partial blocks slice [384:384+N)
    kk = np.arange(128)[:, None]
    cc = np.arange(1024)[None, :]
    m01x = ((cc - 384) >= kk).astype(np.float32)
    onesA = np.ones((128, 1), dtype=np.float32)
    onesB = np.ones((1, 128), dtype=np.float32)

    in_maps = []
    for c in range(NCORES):
        hs = slice(c * DPC, (c + 1) * DPC)
        in_maps.append(
            {
                "xT": xT,
                "wqT": np.ascontiguousarray(wq[hs, :].T),
                "wkT": np.ascontiguousarray(wk[hs, :].T),
                "wvT": np.ascontiguousarray(wv[hs, :].T),
                "woT": np.ascontiguousarray(wo[:, hs].T),
                "m01x": m01x,
                "onesA": onesA,
                "onesB": onesB,
            }
        )

    res = run_bass_kernel_spmd(nc, in_maps, list(range(NCORES)))
    global LAST_RESULT
    LAST_RESULT = res
    acc = res.results[0]["outp"].astype(np.float32)
    for c in range(1, NCORES):
        acc += res.results[c]["outp"]
    # outp is out.T: [m, s_glob] -> [B, S, DIM]
    return np.ascontiguousarray(acc.T).reshape(B, S, DIM)


if __name__ == "__main__":
    rng = np.random.default_rng(0)
    x = rng.standard_normal((B, S, DIM), dtype=np.float32)
    neg = np.float32(-1e9)
    maskm = np.triu(np.full((S, S), neg, dtype=np.float32), k=1)[None, None]
    ws = [rng.standard_normal((DIM, DIM), dtype=np.float32) * 0.02 for _ in range(4)]
    out = kernel(x, maskm, *ws)
    print(out.shape, out.dtype)


# revision 8
# speedup vs baseline: 1.4983x; 1.0641x over previous
"""TRN2 Bass kernel for nn_Attention (B=2, S=2048, DIM=2048, 16 heads).

Sharding: tensor-parallel over heads — 8 cores x 2 heads each.
Each core computes q/k/v projections for its 2 heads over both batches,
causal attention, and a partial output projection (row-parallel wo).
Host sums the 8 partial outputs.

Layouts (per core):
  xT   [2048(k), 4096(s)]  = x.reshape(4096,2048).T          (replicated)
  wqT  [2048(k), 256(dq)]  = wq[head rows].T                  (sharded)
  wkT, wvT likewise; woT [256(dc), 2048(m)] = wo[:, head cols].T
  outp [2048(m), 4096(s)]  partial of out.T                   (summed on host)

All matmuls run in float32r (PE rounds operands to 11 mantissa bits, full
bf16-rate). Set _USE_F32R = False for exact-fp32 (4x slower matmuls).
"""

import sys

sys.path.insert(0, "/opt/trn_rl_repo")

import numpy as np

DIM = 2048
HEADS = 16
HD = 128
B = 2
S = 2048
SG = B * S  # 4096 global sequence (batch-major)
NCORES = 8
HPC = HEADS // NCORES  # 2 heads per core
DPC = HPC * HD  # 256 dims per core
KC = DIM // 128  # 16 contraction chunks
PC = 256  # projection s-chunk width
NPC = S // PC  # 8 proj chunks per batch
AC = 512  # attention sq-chunk width
NAC = S // AC  # 4 attention chunks per batch
ISQ = 1.0 / np.sqrt(np.float32(HD))

_USE_F32R = True

_prog_cache = {}


def _build_program():
    import concourse.bass as bass
    from concourse import bacc
    import concourse.mybir as mybir
    import concourse.tile as tile

    f32 = mybir.dt.float32
    fr = mybir.dt.float32r if _USE_F32R else mybir.dt.float32
    EXP = mybir.ActivationFunctionType.Exp
    LOG = mybir.ActivationFunctionType.Ln

    nc = bacc.Bacc()

    xS = nc.dram_tensor("xS", [SG // PC, 128, KC, PC], fr, kind="ExternalInput")
    wqT = nc.dram_tensor("wqT", [DIM, DPC], fr, kind="ExternalInput")
    wkT = nc.dram_tensor("wkT", [DIM, DPC], fr, kind="ExternalInput")
    wvT = nc.dram_tensor("wvT", [DIM, DPC], fr, kind="ExternalInput")
    woT = nc.dram_tensor("woT", [DPC, DIM], fr, kind="ExternalInput")
    m01x = nc.dram_tensor("m01x", [128, 1024], fr, kind="ExternalInput")
    onesA = nc.dram_tensor("onesA", [128, 1], fr, kind="ExternalInput")
    onesB = nc.dram_tensor("onesB", [1, 128], fr, kind="ExternalInput")
    outp = nc.dram_tensor("outp", [DIM, SG], f32, kind="ExternalOutput")

    with tile.TileContext(nc) as tc:
        with (
            tc.tile_pool(name="wpool", bufs=1) as wpool,
            tc.tile_pool(name="xpool", bufs=2) as xpool,
            tc.tile_pool(name="kv", bufs=1) as kvpool,
            tc.tile_pool(name="work", bufs=2) as work,
            tc.tile_pool(name="expool", bufs=3) as expool,
            tc.tile_pool(name="ps", bufs=1, space="PSUM") as ps,
        ):
            # --- resident constants / weights ---
            wqr = wpool.tile([128, KC, DPC], fr, tag="wqr")
            wkr = wpool.tile([128, KC, DPC], fr, tag="wkr")
            wvr = wpool.tile([128, KC, DPC], fr, tag="wvr")
            wor = wpool.tile([128, HPC, DIM], fr, tag="wor")
            m01 = wpool.tile([128, 1024], fr, tag="m01")
            onA = wpool.tile([128, 1], fr, tag="onA")
            onB = wpool.tile([1, 128], fr, tag="onB")
            def emit_weight_dmas():
                for kc in range(KC):
                    ksl = slice(kc * 128, (kc + 1) * 128)
                    nc.sync.dma_start(wqr[:, kc, :], wqT[ksl, :])
                    nc.sync.dma_start(wkr[:, kc, :], wkT[ksl, :])
                    nc.sync.dma_start(wvr[:, kc, :], wvT[ksl, :])
                nc.sync.dma_start(onA[:], onesA[:])
                nc.sync.dma_start(onB[:], onesB[:])
                nc.sync.dma_start(m01[:], m01x[:])
                for dc in range(HPC):
                    nc.sync.dma_start(
                        wor[:, dc, :], woT[dc * 128 : (dc + 1) * 128, :]
                    )

            # resident per-core activations
            kTr = kvpool.tile([128, B * HPC, S], fr, tag="kTr")  # [d, bh, s]
            vr = kvpool.tile([128, B * (S // 128), DPC], fr, tag="vr")  # [s%, blk, d]

            def proj_units(b, j, qTc):
                dmas = []
                units = []
                for half in range(AC // PC):
                    cl = (AC // PC) * j + half
                    sg0 = b * S + cl * PC
                    xa = xpool.tile(
                        [128, KC, PC], fr, tag="xa", name=f"xa_{b}_{j}_{half}"
                    )

                    cg = b * NPC + cl

                    def dma_unit(xa=xa, cg=cg):
                        nc.sync.dma_start(xa[:], xS[cg])

                    dmas.append(dma_unit)
                    for h in range(HPC):
                        def q_unit(h=h, xa=xa, half=half):
                            dsl = slice(h * 128, (h + 1) * 128)
                            pq = ps.tile([128, PC], f32, tag="pq", bufs=2)
                            for kc in range(KC):
                                nc.tensor.matmul(
                                    pq[:], wqr[:, kc, dsl], xa[:, kc, :],
                                    start=(kc == 0), stop=(kc == KC - 1),
                                )
                            nc.vector.tensor_copy(
                                qTc[:, h, half * PC : (half + 1) * PC], pq[:]
                            )

                        def k_unit(h=h, xa=xa, cl=cl):
                            dsl = slice(h * 128, (h + 1) * 128)
                            pk = ps.tile([128, PC], f32, tag="pq", bufs=2)
                            for kc in range(KC):
                                nc.tensor.matmul(
                                    pk[:], wkr[:, kc, dsl], xa[:, kc, :],
                                    start=(kc == 0), stop=(kc == KC - 1),
                                )
                            nc.vector.tensor_copy(
                                kTr[:, b * HPC + h, cl * PC : (cl + 1) * PC], pk[:]
                            )

                        units.append(q_unit)
                        units.append(k_unit)
                    for sb in range(PC // 128):
                        def v_unit(sb=sb, xa=xa, cl=cl):
                            pv = ps.tile([128, DPC], f32, tag="pq", bufs=2)
                            for kc in range(KC):
                                nc.tensor.matmul(
                                    pv[:], xa[:, kc, sb * 128 : (sb + 1) * 128],
                                    wvr[:, kc, :],
                                    start=(kc == 0), stop=(kc == KC - 1),
                                )
                            vblk = b * (S // 128) + cl * (PC // 128) + sb
                            nc.vector.tensor_copy(vr[:, vblk, :], pv[:])

                        units.append(v_unit)
                return dmas + units

            def att_units(b, j, qTc, uS):
                units = []
                for h in range(HPC):
                    bh = b * HPC + h
                    nblocks = (j + 1) * (AC // 128)
                    nfull = j * (AC // 128)
                    box = {}

                    def head_start(box=box, h=h):
                        box["U"] = ps.tile([128, AC], f32, tag="u", bufs=2,
                                           name=f"U_{b}_{j}_{h}")
                        box["se"] = ps.tile([1, AC], f32, tag="se", bufs=1,
                                            name=f"se_{b}_{j}_{h}")

                    for i in range(nblocks):
                        def block_unit(i=i, h=h, bh=bh, box=box,
                                       nblocks=nblocks, nfull=nfull):
                            if i == 0:
                                head_start(box, h)
                            U, se = box["U"], box["se"]
                            loc = max(0, 128 * i - AC * j)
                            sc = ps.tile([128, AC], f32, tag="sc", bufs=2)
                            ex = expool.tile([128, AC], fr, tag="ex", bufs=4)
                            nc.tensor.matmul(
                                sc[:, loc:AC],
                                kTr[:, bh, i * 128 : (i + 1) * 128],
                                qTc[:, h, loc:AC],
                                start=True, stop=True,
                            )
                            if i < nfull:
                                nc.scalar.activation(ex[:], sc[:], EXP, scale=ISQ)
                            else:
                                ds = expool.tile([128, AC], fr, tag="ds", bufs=2)
                                nc.scalar.activation(
                                    ds[:, loc:AC], sc[:, loc:AC], EXP, scale=ISQ
                                )
                                nc.vector.tensor_mul(
                                    ex[:, loc:AC], ds[:, loc:AC],
                                    m01[:, 384 : 384 + AC - loc],
                                )
                            vblk = b * (S // 128) + i
                            nc.tensor.matmul(
                                U[:, loc:AC],
                                vr[:, vblk, h * 128 : (h + 1) * 128],
                                ex[:, loc:AC],
                                start=(i == 0), stop=(i == nblocks - 1),
                            )
                            nc.tensor.matmul(
                                se[:, loc:AC], onA[:], ex[:, loc:AC],
                                start=(i == 0), stop=(i == nblocks - 1),
                            )

                        units.append(block_unit)

                    def norm_unit(h=h, box=box):
                        U, se = box["U"], box["se"]
                        lnz = work.tile([1, AC], fr, tag="lnz")
                        nc.scalar.activation(lnz[:], se[:], LOG)
                        bc = ps.tile([128, AC], f32, tag="sc", bufs=2)
                        nc.tensor.matmul(bc[:], onB[:], lnz[:], start=True, stop=True)
                        rb = work.tile([128, AC], f32, tag="rb")
                        nc.scalar.activation(rb[:], bc[:], EXP, scale=-1.0)
                        nc.vector.tensor_mul(uS[:, h, :], U[:], rb[:])

                    units.append(norm_unit)
                return units

            def out_units(b, j, uS):
                units = []
                sg0 = b * S + j * AC
                for mb in range(DIM // 128):
                    def o_unit(mb=mb):
                        po = ps.tile([128, AC], f32, tag="po", bufs=1)
                        for dc in range(HPC):
                            nc.tensor.matmul(
                                po[:],
                                wor[:, dc, mb * 128 : (mb + 1) * 128],
                                uS[:, dc, :],
                                start=(dc == 0), stop=(dc == HPC - 1),
                            )
                        ob = work.tile([128, AC], f32, tag="ob")
                        if mb % 2 == 0:
                            nc.vector.tensor_copy(ob[:], po[:])
                        else:
                            nc.scalar.copy(ob[:], po[:])
                        nc.sync.dma_start(
                            outp[mb * 128 : (mb + 1) * 128, sg0 : sg0 + AC], ob[:]
                        )

                    units.append(o_unit)
                return units

            def merge_emit(a_units, b_units):
                na, nb = len(a_units), len(b_units)
                ia = ib = 0
                while ia < na or ib < nb:
                    fa = ia / na if na else 2.0
                    fb = ib / nb if nb else 2.0
                    if fa <= fb:
                        a_units[ia]()
                        ia += 1
                    else:
                        b_units[ib]()
                        ib += 1

            # software pipeline: att(c) interleaved with proj(c+1) + out(c-1)
            chunks = [(b, j) for b in range(B) for j in range(NAC)]
            qTcs = {}
            uSs = {}
            qTcs[chunks[0]] = work.tile([128, HPC, AC], fr, tag="qTc", name="qTc0")
            u0 = proj_units(*chunks[0], qTcs[chunks[0]])
            u0[0]()
            u0[1]()
            emit_weight_dmas()
            for u in u0[2:]:
                u()
            for idx, (b, j) in enumerate(chunks):
                fill = []
                if idx + 1 < len(chunks):
                    nb_, nj_ = chunks[idx + 1]
                    qTcs[(nb_, nj_)] = work.tile(
                        [128, HPC, AC], fr, tag="qTc", name=f"qTc_{nb_}_{nj_}"
                    )
                    fill += proj_units(nb_, nj_, qTcs[(nb_, nj_)])
                if idx > 0:
                    fill += out_units(*chunks[idx - 1], uSs.pop(chunks[idx - 1]))
                uS = work.tile([128, HPC, AC], fr, tag="uS", name=f"uS_{b}_{j}")
                uSs[(b, j)] = uS
                merge_emit(att_units(b, j, qTcs.pop((b, j)), uS), fill)
            for u in out_units(*chunks[-1], uSs.pop(chunks[-1])):
                u()

    nc.finalize()
    return nc


def _get_program():
    key = "prog"
    if key not in _prog_cache:
        _prog_cache[key] = _build_program()
    return _prog_cache[key]


def _is_causal_neg_mask(mask):
    m = mask.reshape(S, S)
    tri = np.triu(np.ones((S, S), dtype=bool), k=1)
    return (
        np.all(m[~tri] == 0.0)
        and np.all(m[tri] <= -1e8)
        and np.all(np.isfinite(m) | tri)
    )


def _reference_fallback(x, mask, wq, wk, wv, wo):
    xf = x.astype(np.float32)
    q = (xf @ wq.T).reshape(B, S, HEADS, HD).transpose(0, 2, 1, 3)
    k = (xf @ wk.T).reshape(B, S, HEADS, HD).transpose(0, 2, 1, 3)
    v = (xf @ wv.T).reshape(B, S, HEADS, HD).transpose(0, 2, 1, 3)
    scores = np.einsum("bhqd,bhkd->bhqk", q, k) / np.sqrt(np.float32(HD))
    scores = scores + mask
    scores = scores - scores.max(axis=-1, keepdims=True)
    e = np.exp(scores)
    probs = e / e.sum(axis=-1, keepdims=True)
    out = np.einsum("bhqk,bhkd->bhqd", probs, v)
    out = out.transpose(0, 2, 1, 3).reshape(B, S, HEADS * HD)
    return (out @ wo.T).astype(np.float32)


def kernel(x, mask, wq, wk, wv, wo):
    x = np.ascontiguousarray(np.asarray(x, dtype=np.float32))
    mask = np.asarray(mask, dtype=np.float32)
    wq = np.ascontiguousarray(np.asarray(wq, dtype=np.float32))
    wk = np.ascontiguousarray(np.asarray(wk, dtype=np.float32))
    wv = np.ascontiguousarray(np.asarray(wv, dtype=np.float32))
    wo = np.ascontiguousarray(np.asarray(wo, dtype=np.float32))

    if not _is_causal_neg_mask(mask):
        return _reference_fallback(x, mask, wq, wk, wv, wo)

    from concourse.bass_utils import run_bass_kernel_spmd

    nc = _get_program()

    xT = x.reshape(SG, DIM).T  # [DIM, SG]
    # xS[cg, p, kc, s'] = xT[kc*128+p, cg*PC+s'] (contiguous per chunk)
    xS = np.ascontiguousarray(
        xT.reshape(KC, 128, SG // PC, PC).transpose(2, 1, 0, 3)
    )
    # m01big[k, c] = 1.0 iff (c - 384) >= k; partial blocks slice [384:384+N)
    kk = np.arange(128)[:, None]
    cc = np.arange(1024)[None, :]
    m01x = ((cc - 384) >= kk).astype(np.float32)
    onesA = np.ones((128, 1), dtype=np.float32)
    onesB = np.ones((1, 128), dtype=np.float32)

    in_maps = []
    for c in range(NCORES):
        hs = slice(c * DPC, (c + 1) * DPC)
        in_maps.append(
            {
                "xS": xS,
                "wqT": np.ascontiguousarray(wq[hs, :].T),
                "wkT": np.ascontiguousarray(wk[hs, :].T),
                "wvT": np.ascontiguousarray(wv[hs, :].T),
                "woT": np.ascontiguousarray(wo[:, hs].T),
                "m01x": m01x,
                "onesA": onesA,
                "onesB": onesB,
            }
        )

    res = run_bass_kernel_spmd(nc, in_maps, list(range(NCORES)))
    global LAST_RESULT
    LAST_RESULT = res
    acc = res.results[0]["outp"].astype(np.float32)
    for c in range(1, NCORES):
        acc += res.results[c]["outp"]
    # outp is out.T: [m, s_glob] -> [B, S, DIM]
    return np.ascontiguousarray(acc.T).reshape(B, S, DIM)


if __name__ == "__main__":
    rng = np.random.default_rng(0)
    x = rng.standard_normal((B, S, DIM), dtype=np.float32)
    neg = np.float32(-1e9)
    maskm = np.triu(np.full((S, S), neg, dtype=np.float32), k=1)[None, None]
    ws = [rng.standard_normal((DIM, DIM), dtype=np.float32) * 0.02 for _ in range(4)]
    out = kernel(x, maskm, *ws)
    print(out.shape, out.dtype)
